# revision 27
# baseline (speedup 1.0000x reference)
"""Trainium2 Bass kernel for per-sample dynamic 3x3 conv (periodic padding).

y[b,o,h,w] = sum_{c,i,j} x[b,c,(h+i-1)%H,(w+j-1)%W] * wgt[b, c*9+i*3+j, o] + bias[b,o]

Shapes: x [16,64,128,128] f32, wgt [16,576,64] f32, bias [16,64] f32.

Sharding: data-parallel over batch, 2 samples per core on 8 cores.

Compute scheme: both per-core samples are packed into single 128x128
matmuls with block-diagonal stationary weights:
  lhsT[k,m] = W_s0[c,o] at (k=c, m=o), W_s1[c,o] at (k=64+c, m=64+o), else 0
  rhs[k,n]  = col-padded img_s0[c, pos] (k<64) / img_s1[c, pos] (k>=64)
so one matmul per 3x3 shift contracts C=64 for both samples at once
(full K=128, full M=128, N=512). Matmuls are float32r (1 cycle/row).

Data movement (descriptor-count bound on this part, so): images load
CONTIGUOUSLY into a raw SBUF tile (16KB runs); the column-wrap-padded
image [128, 128, 130] is built on-chip by DVE/ACT/GPSIMD copies; the
row wrap is handled by splitting the affected matmuls on the two
boundary spatial tiles. Output rows are staged 8 at a time so stores
are 4KB-contiguous-per-partition DMAs.
"""

import numpy as np

KH = KW = 3
B, C, O, H, W = 16, 64, 64, 128, 128
N_CORES = 8
BPC = B // N_CORES  # samples per core
WP = W + 2  # 130: column-wrap padded row length
TILE_ROWS = 4  # output rows per PSUM tile -> N = 4*128 = 512
N_TILES = H // TILE_ROWS
LOAD_CHUNK = 32  # image rows per interior load DMA / pad-build chunk
OGROUP = 8  # spatial tiles per output store group (32 rows)

_CACHE = {}


def _patch_tile_drain():
    """This container's walrus rejects Drain instructions carrying more than
    one sem wait (setupSyncWait: Too many sync wait commands). Re-emit the
    TileContext exit drain's waits as individual wait_ge instructions."""
    import concourse.tile as tile
    from concourse.vector_clock import ScopedClock

    if getattr(tile.TileContext, "_drain_patch_applied", False):
        return

    def _drain_and_barrier(self, tick_clock, wait_clock):
        nc = self.nc
        nop = nc.sync.nop(nofuse=True)
        wait_clock.add_sem_waits(nop.ins, ScopedClock({None: tick_clock.global_clock}))
        waits = list(nop.ins.sync_info.on_wait)
        nop.ins.sync_info.on_wait.clear()
        assert self.sems is not None
        by_name = {}
        for h in self.sems.allocated().values():
            by_name[getattr(h, "name", None)] = h
        for w in waits:
            h = by_name.get(w.ant_name)
            assert h is not None, f"no sem handle for {w.ant_name}"
            nc.sync.wait_ge(h, w.wait_value)
        nc.sync.drain()
        nc.all_engine_barrier()
        popped = nc._tile_sem_poison_stack.pop()
        assert popped is self._sem_poison
        nc.clear_and_free_semaphores(list(self.sems.allocated().values()))
        nc.all_engine_barrier()

    tile.TileContext._drain_and_barrier = _drain_and_barrier
    tile.TileContext._drain_patch_applied = True


def _split_multi_waits(nc, max_waits=1):
    """Same walrus limitation, general form: any instruction carrying more
    than one sem wait fails setupSyncWait. Hoist excess waits onto dedicated
    single-wait NOPs on the same engine, placed just before the instruction."""
    import concourse.mybir as mybir

    for f in nc.m.functions:
        for blk in f.blocks:
            out = []
            changed = False
            for inst in blk.instructions:
                si = getattr(inst, "sync_info", None)
                waits = list(si.on_wait) if si is not None else []
                if len(waits) > max_waits:
                    changed = True
                    for w in waits[:-max_waits]:
                        out.append(
                            mybir.InstNoOp(
                                name=nc.get_next_instruction_name(),
                                engine=inst.engine,
                                sync_info=mybir.SyncInfo(on_wait=[w], on_update=[]),
                                bass_nofuse=True,
                            )
                        )
                    si.on_wait.clear()
                    for w in waits[-max_waits:]:
                        si.on_wait.append(w)
                out.append(inst)
            if changed:
                blk.instructions = out


def _build_module():
    import concourse.bass as bass
    import concourse.mybir as mybir
    import concourse.tile as tile

    _patch_tile_drain()

    f32 = mybir.dt.float32
    f32r = mybir.dt.float32r

    nc = bass.Bass()
    # input/weight feed FP32r matmuls; the BIR verifier requires every
    # producer in that dataflow to be float32r-typed, so declare the whole
    # chain float32r. float32r is byte-identical to float32 host-side.
    x_d = nc.dram_tensor("input", [BPC, C, H, W], f32r, kind="ExternalInput")
    # block-diag weights are pre-assembled host-side in _in_maps:
    # wbd[p, s, m] with wbd[c, s, o] = W_s0, wbd[64+c, s, 64+o] = W_s1, 0 else
    w_d = nc.dram_tensor(
        "wbd", [128, KH * KW, 128], f32r, kind="ExternalInput"
    )
    b_d = nc.dram_tensor("bias", [BPC, O], f32, kind="ExternalInput")
    y_d = nc.dram_tensor("out", [BPC, O, H, W], f32, kind="ExternalOutput")

    with tile.TileContext(nc) as tc:
        from contextlib import ExitStack

        ctx = ExitStack()
        with ctx:
            persist = ctx.enter_context(tc.tile_pool(name="persist", bufs=1))
            psum = ctx.enter_context(tc.tile_pool(name="psum", bufs=6, space="PSUM"))
            psum1 = ctx.enter_context(tc.tile_pool(name="psum1", bufs=1, space="PSUM"))
            ostage = ctx.enter_context(tc.tile_pool(name="ostage", bufs=2))

            # --- weights FIRST (every matmul needs them; 590KB, ~4us) ---
            wts = persist.tile([128, KH * KW, 128], f32r)
            nc.sync.dma_start(out=wts, in_=w_d[:, :, :])

            # --- bias + ACT table preload (Identity's act table costs 1.3us
            # on first use; trigger it during the load phase) ---
            bias_sb = persist.tile([128, 1], f32)
            nc.sync.dma_start(
                out=bias_sb,
                in_=b_d.rearrange("b o -> (b o)").rearrange("(p x) -> p x", x=1),
            )
            act_warm = persist.tile([128, 1], f32)
            nc.scalar.activation(
                out=act_warm, in_=bias_sb,
                func=mybir.ActivationFunctionType.Identity, bias=bias_sb,
            )

            # --- raw images, fully contiguous loads: [128 parts, 128*128].
            # Row H-1 loads first (tile 0 reads it through the periodic
            # wrap); then small-to-large chunks so tile-0 compute starts
            # within a few us while the bulk streams in behind it.
            raw = persist.tile([128, H, W], f32r)
            load_rows = [(H - 1, 1), (0, 8), (8, 24), (32, 32), (64, H - 1 - 64)]
            for r0, nr in load_rows:
                for b in range(BPC):
                    p0 = 64 * b
                    nc.sync.dma_start(
                        out=raw[p0 : p0 + 64, r0 : r0 + nr, :],
                        in_=x_d[b, :, r0 : r0 + nr, :],
                    )

            # --- PE warm-up: HAM unthrottles after ~3.4us of sustained PE
            # activity; burn dummy matmuls on the weight tile during the
            # load phase so the real stream starts at 2.4GHz.
            ps_warm = psum1.tile([128, 512], f32)
            for _ in range(16):
                nc.tensor.matmul(
                    ps_warm,
                    lhsT=wts[:, 0, :],
                    rhs=wts[:, 0:4, :].rearrange("p a b -> p (a b)"),
                    start=True,
                    stop=True,
                )

            # --- column-wrap padded image [128, 128, 130], built on-chip.
            # img[c, r, 0] = x[c, r, 127]; img[c, r, 1:129] = x[c, r, :];
            # img[c, r, 129] = x[c, r, 0]. Row wrap is NOT padded (handled by
            # split matmuls on boundary tiles). Spread copies across engines.
            img = persist.tile([128, H, WP], f32r)
            # (rows, engine): DVE is ~6x faster than GpSimd at these strided
            # copies, so DVE takes the early-needed rows; GpSimd only gets
            # rows not consumed until late in the matmul stream.
            # DVE handles every build the matmul stream needs early (it's the
            # fastest and now has no other work); the late rows go to the
            # otherwise-idle GpSimd; ACT gets one chunk before its merges.
            build_rows = [
                ((H - 1, 1), nc.vector),
                ((0, 8), nc.vector),
                ((8, 24), nc.scalar),
                ((32, 32), nc.vector),
                ((64, 32), nc.vector),
                ((96, H - 1 - 96), nc.gpsimd),
            ]

            def eng_copy(e, out, in_):
                if e is nc.scalar:
                    e.activation(
                        out=out, in_=in_, func=mybir.ActivationFunctionType.Copy
                    )
                else:
                    e.tensor_copy(out=out, in_=in_)

            for (r0, nr), e in build_rows:
                r1 = r0 + nr
                eng_copy(e, img[:, r0:r1, 1 : 1 + W], raw[:, r0:r1, :])
                eng_copy(e, img[:, r0:r1, 0], img[:, r0:r1, W])
                eng_copy(e, img[:, r0:r1, WP - 1], img[:, r0:r1, 1])

            # --- main loop: 32 spatial tiles of 4 output rows.
            # Shift row order [1, 0, 2] so the first matmul of each tile is
            # always a full-coverage N=512 one (start=True zeroes the bank).
            def rhs_rows(i, h0):
                # image rows needed by kernel-row i for out rows h0..h0+3
                return h0 + i - 1

            for t in range(N_TILES):
                h0 = t * TILE_ROWS
                ps = psum.tile([128, TILE_ROWS, W], f32)
                mms = []  # (lhsT, out_slice, rhs_ap, late)
                for i in (1, 0, 2):
                    for j in range(KW):
                        r = rhs_rows(i, h0)
                        lhsT = wts[:, i * KW + j, :]
                        if r < 0:
                            # t=0, i=0: out row 0 reads image row H-1
                            # (loaded+built first, so no reordering needed)
                            mms.append(
                                (lhsT, ps[:, 0:1, :], img[:, H - 1 : H, j : j + W], 0)
                            )
                            mms.append(
                                (lhsT, ps[:, 1:TILE_ROWS, :],
                                 img[:, 0 : TILE_ROWS - 1, j : j + W], 0)
                            )
                        elif r + TILE_ROWS > H:
                            # t=31, i=2: out row 3 reads image row 0
                            mms.append(
                                (lhsT, ps[:, 0 : TILE_ROWS - 1, :],
                                 img[:, r : H, j : j + W], 0)
                            )
                            mms.append(
                                (lhsT, ps[:, TILE_ROWS - 1 : TILE_ROWS, :],
                                 img[:, 0:1, j : j + W], 0)
                            )
                        else:
                            mms.append(
                                (lhsT, ps[:, :, :],
                                 img[:, r : r + TILE_ROWS, j : j + W], 0)
                            )
                mms.sort(key=lambda m: m[3])
                for n, (lhsT, out_sl, rhs, _late) in enumerate(mms):
                    nc.tensor.matmul(
                        out_sl,
                        lhsT=lhsT,
                        rhs=rhs,
                        start=(n == 0),
                        stop=(n == len(mms) - 1),
                    )

                # bias merge into a 32-row staging tile (stores are then 8KB
                # contiguous per partition -> 64 descriptors per DMA).
                # Alternate ACT and DVE so neither engine serializes the PE.
                g = t % OGROUP
                if g == 0:
                    st = ostage.tile([128, OGROUP * TILE_ROWS, W], f32)
                row0 = g * TILE_ROWS
                # merges live on ACT only: DVE must stay free for the image
                # builds, or the scheduler interleaves merges ahead of them
                # and the matmul stream stalls waiting for image rows.
                nc.scalar.activation(
                    out=st[:, row0 : row0 + TILE_ROWS, :],
                    in_=ps,
                    func=mybir.ActivationFunctionType.Identity,
                    bias=bias_sb,
                )
                if g == OGROUP - 1:
                    g0 = (t - OGROUP + 1) * TILE_ROWS
                    for b in range(BPC):
                        nc.sync.dma_start(
                            out=y_d[b, :, g0 : g0 + OGROUP * TILE_ROWS, :],
                            in_=st[64 * b : 64 * b + 64],
                        )
    return nc


def _get_module():
    if "nc" not in _CACHE:
        nc = _build_module()
        # CoreSim can't run modules with post-inserted instructions, so the
        # wait split is applied only on the hardware path.
        _split_multi_waits(nc)
        _CACHE["nc"] = nc
    return _CACHE["nc"]


def _in_maps(input, weight, bias):
    maps = []
    for i in range(N_CORES):
        lo, hi = i * BPC, (i + 1) * BPC
        # prebuild block-diag weights: wbd[64b+c, s, 64b+o] = w[b, c*9+s, o]
        wbd = np.zeros((128, KH * KW, 128), np.float32)
        wloc = weight[lo:hi].reshape(BPC, C, KH * KW, O)
        for b in range(BPC):
            wbd[64 * b : 64 * b + 64, :, 64 * b : 64 * b + 64] = wloc[b]
        maps.append(
            {
                "input": np.ascontiguousarray(input[lo:hi]),
                "wbd": wbd,
                "bias": np.ascontiguousarray(bias[lo:hi]),
            }
        )
    return maps


def kernel(input, weight, bias):
    from concourse.bass_utils import run_bass_kernel_spmd

    nc = _get_module()
    res = run_bass_kernel_spmd(
        nc, _in_maps(input, weight, bias), core_ids=list(range(N_CORES))
    )
    return np.concatenate([res.results[i]["out"] for i in range(N_CORES)], axis=0)


# revision 30
# speedup vs baseline: 1.2635x; 1.2635x over previous
"""Trainium2 Bass kernel for per-sample dynamic 3x3 conv (periodic padding).

y[b,o,h,w] = sum_{c,i,j} x[b,c,(h+i-1)%H,(w+j-1)%W] * wgt[b, c*9+i*3+j, o] + bias[b,o]

Shapes: x [16,64,128,128] f32, wgt [16,576,64] f32, bias [16,64] f32.

Sharding: data-parallel over batch, 2 samples per core on 8 cores.

Compute scheme: both per-core samples are packed into single 128x128
matmuls with block-diagonal stationary weights:
  lhsT[k,m] = W_s0[c,o] at (k=c, m=o), W_s1[c,o] at (k=64+c, m=64+o), else 0
  rhs[k,n]  = col-padded img_s0[c, pos] (k<64) / img_s1[c, pos] (k>=64)
so one matmul per 3x3 shift contracts C=64 for both samples at once
(full K=128, full M=128, N=512). Matmuls are float32r (1 cycle/row).

Data movement (descriptor-count bound on this part, so): images load
CONTIGUOUSLY into a raw SBUF tile (16KB runs); the column-wrap-padded
image [128, 128, 130] is built on-chip by DVE/ACT/GPSIMD copies; the
row wrap is handled by splitting the affected matmuls on the two
boundary spatial tiles. Output rows are staged 8 at a time so stores
are 4KB-contiguous-per-partition DMAs.
"""

import numpy as np

KH = KW = 3
B, C, O, H, W = 16, 64, 64, 128, 128
N_CORES = 8
BPC = B // N_CORES  # samples per core
WP = W + 2  # 130: column-wrap padded row length
TILE_ROWS = 4  # output rows per PSUM tile -> N = 4*128 = 512
N_TILES = H // TILE_ROWS
LOAD_CHUNK = 32  # image rows per interior load DMA / pad-build chunk
OGROUP = 8  # spatial tiles per output store group (32 rows)

_CACHE = {}


def _patch_tile_drain():
    """This container's walrus rejects Drain instructions carrying more than
    one sem wait (setupSyncWait: Too many sync wait commands). Re-emit the
    TileContext exit drain's waits as individual wait_ge instructions."""
    import concourse.tile as tile
    from concourse.vector_clock import ScopedClock

    if getattr(tile.TileContext, "_drain_patch_applied", False):
        return

    def _drain_and_barrier(self, tick_clock, wait_clock):
        nc = self.nc
        nop = nc.sync.nop(nofuse=True)
        wait_clock.add_sem_waits(nop.ins, ScopedClock({None: tick_clock.global_clock}))
        waits = list(nop.ins.sync_info.on_wait)
        nop.ins.sync_info.on_wait.clear()
        assert self.sems is not None
        by_name = {}
        for h in self.sems.allocated().values():
            by_name[getattr(h, "name", None)] = h
        for w in waits:
            h = by_name.get(w.ant_name)
            assert h is not None, f"no sem handle for {w.ant_name}"
            nc.sync.wait_ge(h, w.wait_value)
        nc.sync.drain()
        nc.all_engine_barrier()
        popped = nc._tile_sem_poison_stack.pop()
        assert popped is self._sem_poison
        nc.clear_and_free_semaphores(list(self.sems.allocated().values()))
        nc.all_engine_barrier()

    tile.TileContext._drain_and_barrier = _drain_and_barrier
    tile.TileContext._drain_patch_applied = True


def _split_multi_waits(nc, max_waits=1):
    """Same walrus limitation, general form: any instruction carrying more
    than one sem wait fails setupSyncWait. Hoist excess waits onto dedicated
    single-wait NOPs on the same engine, placed just before the instruction."""
    import concourse.mybir as mybir

    for f in nc.m.functions:
        for blk in f.blocks:
            out = []
            changed = False
            for inst in blk.instructions:
                si = getattr(inst, "sync_info", None)
                waits = list(si.on_wait) if si is not None else []
                if len(waits) > max_waits:
                    changed = True
                    for w in waits[:-max_waits]:
                        out.append(
                            mybir.InstNoOp(
                                name=nc.get_next_instruction_name(),
                                engine=inst.engine,
                                sync_info=mybir.SyncInfo(on_wait=[w], on_update=[]),
                                bass_nofuse=True,
                            )
                        )
                    si.on_wait.clear()
                    for w in waits[-max_waits:]:
                        si.on_wait.append(w)
                out.append(inst)
            if changed:
                blk.instructions = out


def _build_module():
    import concourse.bass as bass
    import concourse.mybir as mybir
    import concourse.tile as tile

    _patch_tile_drain()

    f32 = mybir.dt.float32
    f32r = mybir.dt.float32r

    nc = bass.Bass()
    # input/weight feed FP32r matmuls; the BIR verifier requires every
    # producer in that dataflow to be float32r-typed, so declare the whole
    # chain float32r. float32r is byte-identical to float32 host-side.
    x_d = nc.dram_tensor("input", [BPC, C, H, W], f32r, kind="ExternalInput")
    # block-diag weights are pre-assembled host-side in _in_maps:
    # wbd[p, s, m] with wbd[c, s, o] = W_s0, wbd[64+c, s, 64+o] = W_s1, 0 else
    w_d = nc.dram_tensor(
        "wbd", [128, KH * KW, 128], f32r, kind="ExternalInput"
    )
    b_d = nc.dram_tensor("bias", [BPC, O], f32, kind="ExternalInput")
    y_d = nc.dram_tensor("out", [BPC, O, H, W], f32, kind="ExternalOutput")

    with tile.TileContext(nc) as tc:
        from contextlib import ExitStack

        ctx = ExitStack()
        with ctx:
            persist = ctx.enter_context(tc.tile_pool(name="persist", bufs=1))
            psum = ctx.enter_context(tc.tile_pool(name="psum", bufs=6, space="PSUM"))
            psum1 = ctx.enter_context(tc.tile_pool(name="psum1", bufs=1, space="PSUM"))
            ostage = ctx.enter_context(tc.tile_pool(name="ostage", bufs=2))

            # --- weights FIRST (every matmul needs them; 590KB, ~4us) ---
            wts = persist.tile([128, KH * KW, 128], f32r)
            nc.sync.dma_start(out=wts, in_=w_d[:, :, :])

            # --- bias + ACT table preload (Identity's act table costs 1.3us
            # on first use; trigger it during the load phase) ---
            bias_sb = persist.tile([128, 1], f32)
            nc.sync.dma_start(
                out=bias_sb,
                in_=b_d.rearrange("b o -> (b o)").rearrange("(p x) -> p x", x=1),
            )
            act_warm = persist.tile([128, 1], f32)
            nc.scalar.activation(
                out=act_warm, in_=bias_sb,
                func=mybir.ActivationFunctionType.Identity, bias=bias_sb,
            )

            # --- raw images, fully contiguous loads: [128 parts, 128*128].
            # Row H-1 loads first (tile 0 reads it through the periodic
            # wrap); then small-to-large chunks so tile-0 compute starts
            # within a few us while the bulk streams in behind it.
            # Each DMA must span all 128 partitions: partitions map to the 16
            # DMA engines mod-16, so a 64-partition DMA runs each engine at
            # half throughput (measured 13 vs 26.5 GB/s per engine).
            raw = persist.tile([128, H, W], f32r)
            x_bc = x_d.rearrange("b c h w -> (b c) h w")
            load_rows = [(H - 1, 1), (0, 8), (8, 24), (32, 32), (64, H - 1 - 64)]
            for r0, nr in load_rows:
                nc.sync.dma_start(
                    out=raw[:, r0 : r0 + nr, :],
                    in_=x_bc[:, r0 : r0 + nr, :],
                )

            # --- PE warm-up: HAM unthrottles after ~3.4us of sustained PE
            # activity; burn dummy matmuls on the weight tile during the
            # load phase so the real stream starts at 2.4GHz.
            ps_warm = psum1.tile([128, 512], f32)
            for _ in range(16):
                nc.tensor.matmul(
                    ps_warm,
                    lhsT=wts[:, 0, :],
                    rhs=wts[:, 0:4, :].rearrange("p a b -> p (a b)"),
                    start=True,
                    stop=True,
                )

            # --- column-wrap padded image [128, 128, 130], built on-chip.
            # img[c, r, 0] = x[c, r, 127]; img[c, r, 1:129] = x[c, r, :];
            # img[c, r, 129] = x[c, r, 0]. Row wrap is NOT padded (handled by
            # split matmuls on boundary tiles). Spread copies across engines.
            y_bo = y_d.rearrange("b o h w -> (b o) h w")
            img = persist.tile([128, H, WP], f32r)
            # (rows, engine): DVE is ~6x faster than GpSimd at these strided
            # copies, so DVE takes the early-needed rows; GpSimd only gets
            # rows not consumed until late in the matmul stream.
            # DVE handles every build the matmul stream needs early (it's the
            # fastest and now has no other work); the late rows go to the
            # otherwise-idle GpSimd; ACT gets one chunk before its merges.
            build_rows = [
                ((H - 1, 1), nc.vector),
                ((0, 8), nc.vector),
                ((8, 24), nc.scalar),
                ((32, 32), nc.vector),
                ((64, 32), nc.vector),
                ((96, H - 1 - 96), nc.gpsimd),
            ]

            def eng_copy(e, out, in_):
                if e is nc.scalar:
                    e.activation(
                        out=out, in_=in_, func=mybir.ActivationFunctionType.Copy
                    )
                else:
                    e.tensor_copy(out=out, in_=in_)

            for (r0, nr), e in build_rows:
                r1 = r0 + nr
                eng_copy(e, img[:, r0:r1, 1 : 1 + W], raw[:, r0:r1, :])
                eng_copy(e, img[:, r0:r1, 0], img[:, r0:r1, W])
                eng_copy(e, img[:, r0:r1, WP - 1], img[:, r0:r1, 1])

            # --- main loop: 32 spatial tiles of 4 output rows.
            # Shift row order [1, 0, 2] so the first matmul of each tile is
            # always a full-coverage N=512 one (start=True zeroes the bank).
            def rhs_rows(i, h0):
                # image rows needed by kernel-row i for out rows h0..h0+3
                return h0 + i - 1

            for t in range(N_TILES):
                h0 = t * TILE_ROWS
                ps = psum.tile([128, TILE_ROWS, W], f32)
                mms = []  # (lhsT, out_slice, rhs_ap, late)
                for i in (1, 0, 2):
                    for j in range(KW):
                        r = rhs_rows(i, h0)
                        lhsT = wts[:, i * KW + j, :]
                        if r < 0:
                            # t=0, i=0: out row 0 reads image row H-1
                            # (loaded+built first, so no reordering needed)
                            mms.append(
                                (lhsT, ps[:, 0:1, :], img[:, H - 1 : H, j : j + W], 0)
                            )
                            mms.append(
                                (lhsT, ps[:, 1:TILE_ROWS, :],
                                 img[:, 0 : TILE_ROWS - 1, j : j + W], 0)
                            )
                        elif r + TILE_ROWS > H:
                            # t=31, i=2: out row 3 reads image row 0
                            mms.append(
                                (lhsT, ps[:, 0 : TILE_ROWS - 1, :],
                                 img[:, r : H, j : j + W], 0)
                            )
                            mms.append(
                                (lhsT, ps[:, TILE_ROWS - 1 : TILE_ROWS, :],
                                 img[:, 0:1, j : j + W], 0)
                            )
                        else:
                            mms.append(
                                (lhsT, ps[:, :, :],
                                 img[:, r : r + TILE_ROWS, j : j + W], 0)
                            )
                mms.sort(key=lambda m: m[3])
                for n, (lhsT, out_sl, rhs, _late) in enumerate(mms):
                    nc.tensor.matmul(
                        out_sl,
                        lhsT=lhsT,
                        rhs=rhs,
                        start=(n == 0),
                        stop=(n == len(mms) - 1),
                    )

                # bias merge into a 32-row staging tile (stores are then 8KB
                # contiguous per partition -> 64 descriptors per DMA).
                # Alternate ACT and DVE so neither engine serializes the PE.
                g = t % OGROUP
                if g == 0:
                    st = ostage.tile([128, OGROUP * TILE_ROWS, W], f32)
                row0 = g * TILE_ROWS
                # merges live on ACT only: DVE must stay free for the image
                # builds, or the scheduler interleaves merges ahead of them
                # and the matmul stream stalls waiting for image rows.
                nc.scalar.activation(
                    out=st[:, row0 : row0 + TILE_ROWS, :],
                    in_=ps,
                    func=mybir.ActivationFunctionType.Identity,
                    bias=bias_sb,
                )
                if g == OGROUP - 1:
                    g0 = (t - OGROUP + 1) * TILE_ROWS
                    nc.sync.dma_start(
                        out=y_bo[:, g0 : g0 + OGROUP * TILE_ROWS, :],
                        in_=st,
                    )
    return nc


def _get_module():
    if "nc" not in _CACHE:
        nc = _build_module()
        # CoreSim can't run modules with post-inserted instructions, so the
        # wait split is applied only on the hardware path.
        _split_multi_waits(nc)
        _CACHE["nc"] = nc
    return _CACHE["nc"]


def _in_maps(input, weight, bias):
    maps = []
    for i in range(N_CORES):
        lo, hi = i * BPC, (i + 1) * BPC
        # prebuild block-diag weights: wbd[64b+c, s, 64b+o] = w[b, c*9+s, o]
        wbd = np.zeros((128, KH * KW, 128), np.float32)
        wloc = weight[lo:hi].reshape(BPC, C, KH * KW, O)
        for b in range(BPC):
            wbd[64 * b : 64 * b + 64, :, 64 * b : 64 * b + 64] = wloc[b]
        maps.append(
            {
                "input": np.ascontiguousarray(input[lo:hi]),
                "wbd": wbd,
                "bias": np.ascontiguousarray(bias[lo:hi]),
            }
        )
    return maps


def kernel(input, weight, bias):
    from concourse.bass_utils import run_bass_kernel_spmd

    nc = _get_module()
    res = run_bass_kernel_spmd(
        nc, _in_maps(input, weight, bias), core_ids=list(range(N_CORES))
    )
    return np.concatenate([res.results[i]["out"] for i in range(N_CORES)], axis=0)


# revision 31
# speedup vs baseline: 1.2955x; 1.0253x over previous
"""Trainium2 Bass kernel for per-sample dynamic 3x3 conv (periodic padding).

y[b,o,h,w] = sum_{c,i,j} x[b,c,(h+i-1)%H,(w+j-1)%W] * wgt[b, c*9+i*3+j, o] + bias[b,o]

Shapes: x [16,64,128,128] f32, wgt [16,576,64] f32, bias [16,64] f32.

Sharding: data-parallel over batch, 2 samples per core on 8 cores.

Compute scheme: both per-core samples are packed into single 128x128
matmuls with block-diagonal stationary weights:
  lhsT[k,m] = W_s0[c,o] at (k=c, m=o), W_s1[c,o] at (k=64+c, m=64+o), else 0
  rhs[k,n]  = col-padded img_s0[c, pos] (k<64) / img_s1[c, pos] (k>=64)
so one matmul per 3x3 shift contracts C=64 for both samples at once
(full K=128, full M=128, N=512). Matmuls are float32r (1 cycle/row).

Data movement (descriptor-count bound on this part, so): images load
CONTIGUOUSLY into a raw SBUF tile (16KB runs); the column-wrap-padded
image [128, 128, 130] is built on-chip by DVE/ACT/GPSIMD copies; the
row wrap is handled by splitting the affected matmuls on the two
boundary spatial tiles. Output rows are staged 8 at a time so stores
are 4KB-contiguous-per-partition DMAs.
"""

import numpy as np

KH = KW = 3
B, C, O, H, W = 16, 64, 64, 128, 128
N_CORES = 8
BPC = B // N_CORES  # samples per core
WP = W + 2  # 130: column-wrap padded row length
TILE_ROWS = 4  # output rows per PSUM tile -> N = 4*128 = 512
N_TILES = H // TILE_ROWS
LOAD_CHUNK = 32  # image rows per interior load DMA / pad-build chunk
OGROUP = 8  # spatial tiles per output store group (32 rows)

_CACHE = {}


def _patch_tile_drain():
    """This container's walrus rejects Drain instructions carrying more than
    one sem wait (setupSyncWait: Too many sync wait commands). Re-emit the
    TileContext exit drain's waits as individual wait_ge instructions."""
    import concourse.tile as tile
    from concourse.vector_clock import ScopedClock

    if getattr(tile.TileContext, "_drain_patch_applied", False):
        return

    def _drain_and_barrier(self, tick_clock, wait_clock):
        nc = self.nc
        nop = nc.sync.nop(nofuse=True)
        wait_clock.add_sem_waits(nop.ins, ScopedClock({None: tick_clock.global_clock}))
        waits = list(nop.ins.sync_info.on_wait)
        nop.ins.sync_info.on_wait.clear()
        assert self.sems is not None
        by_name = {}
        for h in self.sems.allocated().values():
            by_name[getattr(h, "name", None)] = h
        for w in waits:
            h = by_name.get(w.ant_name)
            assert h is not None, f"no sem handle for {w.ant_name}"
            nc.sync.wait_ge(h, w.wait_value)
        nc.sync.drain()
        nc.all_engine_barrier()
        popped = nc._tile_sem_poison_stack.pop()
        assert popped is self._sem_poison
        nc.clear_and_free_semaphores(list(self.sems.allocated().values()))

    tile.TileContext._drain_and_barrier = _drain_and_barrier
    tile.TileContext._drain_patch_applied = True


def _split_multi_waits(nc, max_waits=1):
    """Same walrus limitation, general form: any instruction carrying more
    than one sem wait fails setupSyncWait. Hoist excess waits onto dedicated
    single-wait NOPs on the same engine, placed just before the instruction."""
    import concourse.mybir as mybir

    for f in nc.m.functions:
        for blk in f.blocks:
            out = []
            changed = False
            for inst in blk.instructions:
                si = getattr(inst, "sync_info", None)
                waits = list(si.on_wait) if si is not None else []
                if len(waits) > max_waits:
                    changed = True
                    for w in waits[:-max_waits]:
                        out.append(
                            mybir.InstNoOp(
                                name=nc.get_next_instruction_name(),
                                engine=inst.engine,
                                sync_info=mybir.SyncInfo(on_wait=[w], on_update=[]),
                                bass_nofuse=True,
                            )
                        )
                    si.on_wait.clear()
                    for w in waits[-max_waits:]:
                        si.on_wait.append(w)
                out.append(inst)
            if changed:
                blk.instructions = out


def _build_module():
    import concourse.bass as bass
    import concourse.mybir as mybir
    import concourse.tile as tile

    _patch_tile_drain()

    f32 = mybir.dt.float32
    f32r = mybir.dt.float32r

    nc = bass.Bass()
    # input/weight feed FP32r matmuls; the BIR verifier requires every
    # producer in that dataflow to be float32r-typed, so declare the whole
    # chain float32r. float32r is byte-identical to float32 host-side.
    x_d = nc.dram_tensor("input", [BPC, C, H, W], f32r, kind="ExternalInput")
    # block-diag weights are pre-assembled host-side in _in_maps:
    # wbd[p, s, m] with wbd[c, s, o] = W_s0, wbd[64+c, s, 64+o] = W_s1, 0 else
    w_d = nc.dram_tensor(
        "wbd", [128, KH * KW, 128], f32r, kind="ExternalInput"
    )
    b_d = nc.dram_tensor("bias", [BPC, O], f32, kind="ExternalInput")
    y_d = nc.dram_tensor("out", [BPC, O, H, W], f32, kind="ExternalOutput")

    with tile.TileContext(nc) as tc:
        from contextlib import ExitStack

        ctx = ExitStack()
        with ctx:
            persist = ctx.enter_context(tc.tile_pool(name="persist", bufs=1))
            psum = ctx.enter_context(tc.tile_pool(name="psum", bufs=6, space="PSUM"))
            ostage = ctx.enter_context(tc.tile_pool(name="ostage", bufs=2))

            # --- weights FIRST (every matmul needs them; 590KB, ~4us) ---
            wts = persist.tile([128, KH * KW, 128], f32r)
            nc.sync.dma_start(out=wts, in_=w_d[:, :, :])

            # --- bias + ACT table preload (Identity's act table costs 1.3us
            # on first use; trigger it during the load phase) ---
            bias_sb = persist.tile([128, 1], f32)
            nc.sync.dma_start(
                out=bias_sb,
                in_=b_d.rearrange("b o -> (b o)").rearrange("(p x) -> p x", x=1),
            )
            act_warm = persist.tile([128, 1], f32)
            nc.scalar.activation(
                out=act_warm, in_=bias_sb,
                func=mybir.ActivationFunctionType.Identity, bias=bias_sb,
            )

            # --- raw images, fully contiguous loads: [128 parts, 128*128].
            # Row H-1 loads first (tile 0 reads it through the periodic
            # wrap); then small-to-large chunks so tile-0 compute starts
            # within a few us while the bulk streams in behind it.
            # Each DMA must span all 128 partitions: partitions map to the 16
            # DMA engines mod-16, so a 64-partition DMA runs each engine at
            # half throughput (measured 13 vs 26.5 GB/s per engine).
            raw = persist.tile([128, H, W], f32r)
            x_bc = x_d.rearrange("b c h w -> (b c) h w")
            load_rows = [(H - 1, 1), (0, 8), (8, 24), (32, 32), (64, H - 1 - 64)]
            for r0, nr in load_rows:
                nc.sync.dma_start(
                    out=raw[:, r0 : r0 + nr, :],
                    in_=x_bc[:, r0 : r0 + nr, :],
                )


            # --- column-wrap padded image [128, 128, 130], built on-chip.
            # img[c, r, 0] = x[c, r, 127]; img[c, r, 1:129] = x[c, r, :];
            # img[c, r, 129] = x[c, r, 0]. Row wrap is NOT padded (handled by
            # split matmuls on boundary tiles). Spread copies across engines.
            y_bo = y_d.rearrange("b o h w -> (b o) h w")
            img = persist.tile([128, H, WP], f32r)
            # (rows, engine): DVE is ~6x faster than GpSimd at these strided
            # copies, so DVE takes the early-needed rows; GpSimd only gets
            # rows not consumed until late in the matmul stream.
            # DVE handles every build the matmul stream needs early (it's the
            # fastest and now has no other work); the late rows go to the
            # otherwise-idle GpSimd; ACT gets one chunk before its merges.
            build_rows = [
                ((H - 1, 1), nc.vector),
                ((0, 8), nc.vector),
                ((8, 24), nc.scalar),
                ((32, 32), nc.vector),
                ((64, 32), nc.vector),
                ((96, H - 1 - 96), nc.gpsimd),
            ]

            def eng_copy(e, out, in_):
                if e is nc.scalar:
                    e.activation(
                        out=out, in_=in_, func=mybir.ActivationFunctionType.Copy
                    )
                else:
                    e.tensor_copy(out=out, in_=in_)

            for (r0, nr), e in build_rows:
                r1 = r0 + nr
                eng_copy(e, img[:, r0:r1, 1 : 1 + W], raw[:, r0:r1, :])
                eng_copy(e, img[:, r0:r1, 0], img[:, r0:r1, W])
                eng_copy(e, img[:, r0:r1, WP - 1], img[:, r0:r1, 1])

            # --- main loop: 32 spatial tiles of 4 output rows.
            # Shift row order [1, 0, 2] so the first matmul of each tile is
            # always a full-coverage N=512 one (start=True zeroes the bank).
            def rhs_rows(i, h0):
                # image rows needed by kernel-row i for out rows h0..h0+3
                return h0 + i - 1

            for t in range(N_TILES):
                h0 = t * TILE_ROWS
                ps = psum.tile([128, TILE_ROWS, W], f32)
                mms = []  # (lhsT, out_slice, rhs_ap, late)
                for i in (1, 0, 2):
                    for j in range(KW):
                        r = rhs_rows(i, h0)
                        lhsT = wts[:, i * KW + j, :]
                        if r < 0:
                            # t=0, i=0: out row 0 reads image row H-1
                            # (loaded+built first, so no reordering needed)
                            mms.append(
                                (lhsT, ps[:, 0:1, :], img[:, H - 1 : H, j : j + W], 0)
                            )
                            mms.append(
                                (lhsT, ps[:, 1:TILE_ROWS, :],
                                 img[:, 0 : TILE_ROWS - 1, j : j + W], 0)
                            )
                        elif r + TILE_ROWS > H:
                            # t=31, i=2: out row 3 reads image row 0
                            mms.append(
                                (lhsT, ps[:, 0 : TILE_ROWS - 1, :],
                                 img[:, r : H, j : j + W], 0)
                            )
                            mms.append(
                                (lhsT, ps[:, TILE_ROWS - 1 : TILE_ROWS, :],
                                 img[:, 0:1, j : j + W], 0)
                            )
                        else:
                            mms.append(
                                (lhsT, ps[:, :, :],
                                 img[:, r : r + TILE_ROWS, j : j + W], 0)
                            )
                mms.sort(key=lambda m: m[3])
                for n, (lhsT, out_sl, rhs, _late) in enumerate(mms):
                    nc.tensor.matmul(
                        out_sl,
                        lhsT=lhsT,
                        rhs=rhs,
                        start=(n == 0),
                        stop=(n == len(mms) - 1),
                    )

                # bias merge into a 32-row staging tile (stores are then 8KB
                # contiguous per partition -> 64 descriptors per DMA).
                # Alternate ACT and DVE so neither engine serializes the PE.
                g = t % OGROUP
                if g == 0:
                    st = ostage.tile([128, OGROUP * TILE_ROWS, W], f32)
                row0 = g * TILE_ROWS
                # merges live on ACT only: DVE must stay free for the image
                # builds, or the scheduler interleaves merges ahead of them
                # and the matmul stream stalls waiting for image rows.
                nc.scalar.activation(
                    out=st[:, row0 : row0 + TILE_ROWS, :],
                    in_=ps,
                    func=mybir.ActivationFunctionType.Identity,
                    bias=bias_sb,
                )
                if t == N_TILES - 5:
                    # flush the first half of the last group early so the
                    # end-of-kernel store tail is only 16 rows
                    nc.sync.dma_start(
                        out=y_bo[:, 96:112, :], in_=st[:, 0:16, :]
                    )
                if g == OGROUP - 1:
                    g0 = (t - OGROUP + 1) * TILE_ROWS
                    if t == N_TILES - 1:
                        nc.sync.dma_start(
                            out=y_bo[:, 112:128, :], in_=st[:, 16:32, :]
                        )
                    else:
                        nc.sync.dma_start(
                            out=y_bo[:, g0 : g0 + OGROUP * TILE_ROWS, :],
                            in_=st,
                        )
    return nc


def _get_module():
    if "nc" not in _CACHE:
        nc = _build_module()
        # CoreSim can't run modules with post-inserted instructions, so the
        # wait split is applied only on the hardware path.
        _split_multi_waits(nc)
        _CACHE["nc"] = nc
    return _CACHE["nc"]


def _in_maps(input, weight, bias):
    maps = []
    for i in range(N_CORES):
        lo, hi = i * BPC, (i + 1) * BPC
        # prebuild block-diag weights: wbd[64b+c, s, 64b+o] = w[b, c*9+s, o]
        wbd = np.zeros((128, KH * KW, 128), np.float32)
        wloc = weight[lo:hi].reshape(BPC, C, KH * KW, O)
        for b in range(BPC):
            wbd[64 * b : 64 * b + 64, :, 64 * b : 64 * b + 64] = wloc[b]
        maps.append(
            {
                "input": np.ascontiguousarray(input[lo:hi]),
                "wbd": wbd,
                "bias": np.ascontiguousarray(bias[lo:hi]),
            }
        )
    return maps


def kernel(input, weight, bias):
    from concourse.bass_utils import run_bass_kernel_spmd

    nc = _get_module()
    res = run_bass_kernel_spmd(
        nc, _in_maps(input, weight, bias), core_ids=list(range(N_CORES))
    )
    return np.concatenate([res.results[i]["out"] for i in range(N_CORES)], axis=0)
